# revision 20
# baseline (speedup 1.0000x reference)
"""Trainium2 Bass kernel for nn_MCPInitEmbedding (segment_reduce).

Problem: out[b, s, :] = sum_{j<100} (weights[b, idx[b,s,j]] * w + bias)
       = (sum_j weights[b, idx[b,s,j]]) * w + 100 * bias

The gather-sum S[b,s] = sum_j weights[b, idx[b,s,j]] is recast as a
dense matmul against a host-built (index-only preprocessing) counts
matrix C[item, set] in fp8 (counts are exact small ints in fp8):

    S[:, s] = sum_item weights[item] * C[item, set]

On device: 40 accumulating DoubleRow PE matmuls per batch per 512-set
half (fp8 perf mode contracts K=256 items per instruction; DoubleRow
requires 16-aligned k-tile steps AND 16B-aligned operand offsets, so
sets are laid out in 512-wide halves: per pair, 2048 cols =
[h0:[i0|i1]][h1:[i0|i1]], each sub-block 512).  The rank-1 expansion
S*w + 100*b reuses the baseline's K=2 matmul.  The C stream
saturates the per-core HBM bandwidth with 8-pair (16KB/partition)
supertiles — fewer, larger DMAs beat smaller ones (each extra
dma_start costs ~0.4us of ring bubble); the first supertiles taper
up so the PE phase-locks with the DMA stream from the start, and the
last batch's supertiles rebalance [10,10,10,8,2] at constant DMA
count so only 2 pairs of PE work remain after the final transfer.
Output is written [partition, set-tile] contiguous, un-permuted on
host.

Sharding: pure data parallel, 2 batches per core across 8 cores.
"""
import numpy as np
import ml_dtypes

import concourse.bacc as bacc
import concourse.tile as tile
import concourse.mybir as mybir
from concourse.bass_utils import run_bass_kernel_spmd

B = 16
N_ITEMS = 10000
N_SETS = 1000
SET_SZ = 100
D = 128
N_CORES = 8
BPC = B // N_CORES  # batches per core = 2

KCH = 128                       # items per k-tile (partition dim)
NPAIR = 39                      # full DoubleRow pairs (items 0..9983)
NTAIL = N_ITEMS - 2 * NPAIR * KCH  # 16 tail items (9984..9999)
ITEMS_PAD = 80 * KCH            # wcol padding (10240; stride must be %16)
WSTRIDE = BPC * 40              # wcol k-tile stride (80, mult of 16)
HSET = 512                      # padded sets per half (500 real)
PAIR_COLS = 2 * 2 * HSET        # 2048 cm columns per pair
TAIL_COLS = 2 * HSET            # 1024 cm columns for the K=16 tail block
BB_COLS = NPAIR * PAIR_COLS + TAIL_COLS  # cm columns per batch
# pairs per DMA supertile; the first supertiles taper UP so the PE
# starts consuming almost immediately and stays phase-locked with the
# DMA stream (a lagging PE exhausts the stream buffers and stalls DMA).
SUPS = [[2, 2, 4, 10, 10, 11], [14, 13, 10, 2]]
assert [sum(s) for s in SUPS] == [NPAIR] * BPC

F32 = mybir.dt.float32
BF16 = mybir.dt.bfloat16
F8 = mybir.dt.float8e4
NP_F8 = ml_dtypes.float8_e4m3

_CACHED = {}


def _build_program():
    nc = bacc.Bacc("TRN2", target_bir_lowering=False, debug=False,
                   num_devices=N_CORES)
    cm = nc.dram_tensor("cm", [128, BPC * BB_COLS], F8,
                        kind="ExternalInput").ap()
    # aux fp8 bundle: cols [0,160) wcol (k-tile-major, column
    # i*WSTRIDE + bb*40 + pair); cols [160+bb*1024, +1024) tail-count
    # block for batch bb (partitions 0..15); cols [2208, 2210) wtail
    # (wtail[p, bb] = weights[bb, 9984 + p], partitions 0..15)
    aux = nc.dram_tensor("aux", [128, 2 * WSTRIDE + BPC * TAIL_COLS + BPC],
                         F8, kind="ExternalInput").ap()
    wb = nc.dram_tensor("wb", [2, D], BF16, kind="ExternalInput").ap()
    out = nc.dram_tensor("out", [128, BPC * 8 * D], F32,
                         kind="ExternalOutput").ap()

    DR = mybir.MatmulPerfMode.DoubleRow

    with tile.TileContext(nc) as tc:
        with (
            tc.tile_pool(name="small", bufs=1) as pool,
            tc.tile_pool(name="cstream", bufs=4) as cpool,
            tc.tile_pool(name="gp", bufs=2) as gpool,
            tc.tile_pool(name="ps", bufs=2, space="PSUM") as psp,
            tc.tile_pool(name="pse", bufs=2, space="PSUM") as pse,
        ):
            dma_engs = [nc.scalar, nc.sync]
            auxt = pool.tile([128, 2 * WSTRIDE + BPC * TAIL_COLS + BPC], F8)
            wbt = pool.tile([2, D], BF16)
            nc.sync.dma_start(auxt[:], aux)
            nc.sync.dma_start(wbt[:], wb)
            wcolt = auxt

            di = 0
            for bb in range(BPC):
                ps0 = psp.tile([1, HSET], F32, tag="ps0")
                ps1 = psp.tile([1, HSET], F32, tag="ps1")
                ctt_off = 2 * WSTRIDE + bb * TAIL_COLS
                p0 = 0
                for np_ in SUPS[bb]:
                    cols = np_ * PAIR_COLS
                    coff = bb * BB_COLS + p0 * PAIR_COLS
                    ct = cpool.tile([128, 14 * PAIR_COLS], F8, tag="ct")
                    eng = dma_engs[di % len(dma_engs)]
                    di += 1
                    eng.dma_start(ct[:, :cols], cm[:, coff : coff + cols])
                    for pl in range(np_):
                        P = p0 + pl
                        pidx = bb * 40 + P
                        # [128, 2, 1] with k-tile step WSTRIDE (mult of 16)
                        lpair = wcolt[
                            :, pidx : pidx + WSTRIDE + 1 : WSTRIDE
                        ].rearrange("p (i j) -> p i j", j=1)
                        for h, ps in ((0, ps0), (1, ps1)):
                            blk = pl * PAIR_COLS + h * 2 * HSET
                            nc.tensor.matmul(
                                out=ps[0:1, :],
                                lhsT=lpair,
                                rhs=ct[:, blk : blk + 2 * HSET].rearrange(
                                    "p (i j) -> p i j", i=2
                                ),
                                perf_mode=DR,
                                start=(P == 0),
                                stop=False,
                            )
                    p0 += np_
                # last 16 items: ordinary K=16 matmuls close each group
                for h, ps in ((0, ps0), (1, ps1)):
                    nc.tensor.matmul(
                        out=ps[0:1, :],
                        lhsT=auxt[0:NTAIL, 2 * WSTRIDE + BPC * TAIL_COLS
                                  + bb : 2 * WSTRIDE + BPC * TAIL_COLS
                                  + bb + 1],
                        rhs=auxt[0:NTAIL, ctt_off + h * HSET
                                 : ctt_off + (h + 1) * HSET],
                        start=False,
                        stop=True,
                    )

                # srow row0 = set sums, row1 = SET_SZ; K=2 matmul vs [w; b]
                srow = gpool.tile([2, 1024], BF16, tag="srow")
                nc.vector.memset(srow[0:2, :], float(SET_SZ))
                nc.vector.tensor_copy(srow[0:1, 0:500], ps0[0:1, 0:500])
                nc.vector.tensor_copy(srow[0:1, 500:1000], ps1[0:1, 0:500])
                ob = gpool.tile([128, 1024], F32, tag="ob")
                # 4 expansion matmuls per PSUM bank -> one wide DVE copy
                # each (8 small PSUM reads serialized on DVE cost ~2x)
                for g in range(2):
                    psd = pse.tile([128, 4 * D], F32, tag="psd")
                    for mi in range(4):
                        m = g * 4 + mi
                        nc.tensor.matmul(
                            out=psd[:, mi * D : (mi + 1) * D],
                            lhsT=srow[0:2, m * 128 : (m + 1) * 128],
                            rhs=wbt[:],
                            start=True,
                            stop=True,
                        )
                    nc.vector.tensor_copy(
                        ob[:, g * 4 * D : (g + 1) * 4 * D], psd[:]
                    )
                # contiguous store; host un-permutes [p, (bb,m,d)] -> [bb,s,d]
                nc.scalar.dma_start(
                    out[:, bb * 8 * D : (bb + 1) * 8 * D], ob[:, : 8 * D]
                )

    nc.compile()
    return nc


def _counts_fp8(mem_batch: np.ndarray) -> np.ndarray:
    """membership for one batch [1000, 100] int -> fp8 counts
    [128, NPAIR*PAIR_COLS] in DoubleRow layout.

    C[item, set] = multiplicity of item in set's member list.  Column
    layout per pair P (items 256P..256P+255): [half h: [ktile i=0:
    512 cols (500 real sets + 12 zero pad)][ktile i=1: same]] x2.
    """
    sets = np.repeat(np.arange(N_SETS, dtype=np.int64), SET_SZ)
    items = mem_batch.reshape(-1).astype(np.int64)
    cnt = np.bincount(items * N_SETS + sets,
                      minlength=N_ITEMS * N_SETS).astype(np.uint8)
    cnt = cnt.reshape(N_ITEMS, N_SETS)
    # full pairs: dims (P, i, p, h, s')
    cnt5 = cnt[: 2 * NPAIR * KCH].reshape(NPAIR, 2, KCH, 2, 500)
    cp = np.zeros((NPAIR, 2, KCH, 2, HSET), dtype=np.uint8)
    cp[..., :500] = cnt5
    cp = cp.transpose(2, 0, 3, 1, 4)  # -> (p, P, h, i, s')
    full = cp.reshape(128, NPAIR * PAIR_COLS)
    # tail block: items 9984.. on partitions 0..15, [h0: 512][h1: 512]
    tail = np.zeros((128, TAIL_COLS), dtype=np.uint8)
    tc = cnt[2 * NPAIR * KCH :].reshape(NTAIL, 2, 500)
    tail[:NTAIL, :500] = tc[:, 0]
    tail[:NTAIL, HSET : HSET + 500] = tc[:, 1]
    return np.ascontiguousarray(
        np.concatenate([full, tail], axis=1)
    ).astype(NP_F8)


def make_in_maps(weights, membership, w, b):
    weights = np.asarray(weights, dtype=np.float32)
    membership = np.asarray(membership)
    w = np.asarray(w, dtype=np.float32)
    b = np.asarray(b, dtype=np.float32)

    wb_np = np.stack([w, b]).astype(ml_dtypes.bfloat16)  # [2, 128]
    wpad = np.zeros((B, ITEMS_PAD), dtype=np.float32)
    wpad[:, :N_ITEMS] = weights
    in_maps = []
    for core in range(N_CORES):
        cm_np = np.concatenate(
            [_counts_fp8(membership[core * BPC + bb]) for bb in range(BPC)],
            axis=1,
        )
        # wcol[p, i*WSTRIDE + bb*40 + P] = weights[bb, (2P+i)*128 + p]
        wc = np.stack(
            [
                wpad[core * BPC + bb].reshape(40, 2, KCH)
                for bb in range(BPC)
            ]
        )  # [BPC, P, i, p]
        wcol_np = np.ascontiguousarray(
            wc.transpose(3, 2, 0, 1).reshape(128, 2 * WSTRIDE)
        ).astype(NP_F8)
        wtail_np = np.ascontiguousarray(
            weights[core * BPC : (core + 1) * BPC, 2 * NPAIR * KCH :].T
        ).astype(NP_F8)  # [16, BPC]
        aux_np = np.zeros(
            (128, 2 * WSTRIDE + BPC * TAIL_COLS + BPC), dtype=NP_F8
        )
        aux_np[:, : 2 * WSTRIDE] = wcol_np
        for bb in range(BPC):
            aux_np[:, 2 * WSTRIDE + bb * TAIL_COLS
                   : 2 * WSTRIDE + (bb + 1) * TAIL_COLS] = cm_np[
                :, bb * BB_COLS + NPAIR * PAIR_COLS : (bb + 1) * BB_COLS
            ]
        aux_np[:NTAIL, 2 * WSTRIDE + BPC * TAIL_COLS :] = wtail_np
        in_maps.append({"cm": cm_np, "aux": aux_np, "wb": wb_np})
    return in_maps


def _unpermute_out(arr: np.ndarray) -> np.ndarray:
    """[128, BPC*8*128] -> [BPC, 1000, 128]."""
    return (
        arr.reshape(128, BPC, 8, D)
        .transpose(1, 2, 0, 3)
        .reshape(BPC, 8 * 128, D)[:, :N_SETS, :]
    )


def kernel(weights, membership, w, b):
    if "nc" not in _CACHED:
        _CACHED["nc"] = _build_program()
    nc = _CACHED["nc"]

    in_maps = make_in_maps(weights, membership, w, b)
    res = run_bass_kernel_spmd(nc, in_maps, core_ids=list(range(N_CORES)))
    out = np.concatenate(
        [_unpermute_out(res.results[c]["out"]) for c in range(N_CORES)],
        axis=0,
    )
    return np.ascontiguousarray(out).astype(np.float32)
